# revision 5
# baseline (speedup 1.0000x reference)
"""AAM-Softmax (ArcFace) logits kernel for Trainium2, 8 NeuronCores.

Math (per reference):
    cosine = l2norm(input) @ l2norm(weight).T            # [B, C]
    tgt    = cosine[i, label[i]]
    phi    = tgt*cos(m) - sqrt(1-tgt^2)*sin(m)
    out    = S * cosine, except out[i, label[i]] = S * where(tgt>0, phi, tgt)

Sharding: weight/cosine column-sharded over 8 cores (vocab parallel);
input + labels replicated.  Core k owns classes [k*CS, (k+1)*CS).

Per-core device pipeline:
  - x [B, D] f32 -> row sumsq -> xinvS = S/||x|| (and xinv = 1/||x||)
  - xhatS = x * xinvS (bf16), PE-transposed into xT [D, B] bf16
  - wt input is host-relayouted W.T shard [2, 128, CS] f32 (pure relayout,
    no arithmetic).  Per 500-col tile: cast to bf16; square (bf16) and
    ones-matmul -> column sumsq broadcast over partitions in PSUM;
    sqrt + reciprocal -> winv tile [128, 500].
  - main matmul: out_psum[b-tile] = xT.T @ wt_bf (K=256 over 2 chunks)
  - staging = out_psum * winv  (fuses the weight-norm column scale; x side
    already carries S), DMA to out[b-tile, c-tile].
  - margin: w_sel = weight[label] (host gather, replicated input; all
    arithmetic on device): tgt = (x . wsel) * xinv * wselinv; phi/select
    math on [128, 8]; final values scattered into out[i, label_local[i]]
    via indirect DMA (out-of-shard rows get OOB offsets and are skipped).
"""

import sys

if "/opt/trn_rl_repo" not in sys.path:
    sys.path.insert(0, "/opt/trn_rl_repo")

from dataclasses import dataclass

import numpy as np

S = 50.0
MARGIN = 0.5
COS_M = float(np.cos(MARGIN))
SIN_M = float(np.sin(MARGIN))
OOB = 16000000.0  # exact in f32, > any valid flat offset


@dataclass(frozen=True)
class Cfg:
    b: int = 1024
    d: int = 256
    c: int = 100000
    ncores: int = 8
    tc: int = 500

    @property
    def cs(self):
        return self.c // self.ncores

    @property
    def nb(self):
        return self.b // 128

    @property
    def nkt(self):
        return self.d // 128

    @property
    def nct(self):
        return self.cs // self.tc


def build(cfg: Cfg):
    import concourse.bass as bass
    import concourse.tile as tile
    from concourse import bacc, mybir
    from concourse.masks import make_identity

    f32 = mybir.dt.float32
    bf16 = mybir.dt.bfloat16
    i32 = mybir.dt.int32
    X = mybir.AxisListType.X
    Op = mybir.AluOpType
    Act = mybir.ActivationFunctionType

    b, d, cs, tc = cfg.b, cfg.d, cfg.cs, cfg.tc
    nb, nkt, nct = cfg.nb, cfg.nkt, cfg.nct

    nc = bacc.Bacc(
        "TRN2", target_bir_lowering=False, debug=False, num_devices=cfg.ncores
    )

    x_ext = nc.dram_tensor("x", [b, d], f32, kind="ExternalInput")
    wt_ext = nc.dram_tensor("wt", [nkt, 128, cs], f32, kind="ExternalInput")
    wsel_ext = nc.dram_tensor("wsel", [b, d], f32, kind="ExternalInput")
    labrel_ext = nc.dram_tensor("labrel", [128, nb], i32, kind="ExternalInput")
    out_ext = nc.dram_tensor("out", [b, cs], f32, kind="ExternalOutput")

    with tile.TileContext(nc) as tc_:
        with (
            tc_.tile_pool(name="const", bufs=1) as constp,
            tc_.tile_pool(name="persist", bufs=1) as persist,
            tc_.tile_pool(name="xin", bufs=2) as xin,
            tc_.tile_pool(name="xsc", bufs=2) as xsc,
            tc_.tile_pool(name="tiny", bufs=2) as tiny,
            tc_.tile_pool(name="wstream", bufs=4) as wstream,
            tc_.tile_pool(name="wbf", bufs=3) as wbf,
            tc_.tile_pool(name="winvp", bufs=2) as winvp,
            tc_.tile_pool(name="stage", bufs=6) as stage,
            tc_.tile_pool(name="pt", bufs=2, space="PSUM") as pt,
            tc_.tile_pool(name="pn", bufs=2, space="PSUM") as pn,
            tc_.tile_pool(name="po", bufs=4, space="PSUM") as po,
        ):
            ident_bf = constp.tile([128, 128], bf16)
            make_identity(nc, ident_bf[:])
            ones_bf = constp.tile([128, 128], bf16)
            nc.vector.memset(ones_bf[:], 1.0)

            # persistent tensors
            xT = persist.tile([128, nkt * b], bf16)  # [d-half on part][k*b + i]
            labrel_t = persist.tile([128, nb], i32)
            rel_f = persist.tile([128, nb], f32)
            iota_i = persist.tile([128, nb], i32)
            iota_f = persist.tile([128, nb], f32)
            xinv8 = persist.tile([128, nb], f32)
            wsinv8 = persist.tile([128, nb], f32)
            rawdot8 = persist.tile([128, nb], f32)
            newv8 = persist.tile([128, nb], f32)
            offs_i = persist.tile([128, nb], i32)

            nc.sync.dma_start(labrel_t[:], labrel_ext[:])
            # flat row base = p*cs + f*128*cs; iota steps are int16-limited so
            # build p*cs and f separately and combine in f32 (values < 2^24).
            nc.gpsimd.iota(
                iota_i[:], pattern=[[0, nb]], base=0, channel_multiplier=cs
            )
            iota_j = persist.tile([128, nb], i32)
            nc.gpsimd.iota(
                iota_j[:], pattern=[[1, nb]], base=0, channel_multiplier=0
            )
            iotaj_f = persist.tile([128, nb], f32)
            nc.vector.tensor_copy(iota_f[:], iota_i[:])
            nc.vector.tensor_copy(iotaj_f[:], iota_j[:])
            nc.vector.tensor_scalar_mul(iotaj_f[:], iotaj_f[:], float(128 * cs))
            nc.vector.tensor_add(iota_f[:], iota_f[:], iotaj_f[:])
            nc.vector.tensor_copy(rel_f[:], labrel_t[:])

            # ---- Phase A: x prep (+ wsel/tgt path) ----
            for bi in range(nb):
                rsl = slice(bi * 128, (bi + 1) * 128)
                x_t = xin.tile([128, d], f32)
                nc.sync.dma_start(x_t[:], x_ext[rsl, :])
                sq = xsc.tile([128, d], f32)
                nc.vector.tensor_mul(sq[:], x_t[:], x_t[:])
                ss = tiny.tile([128, 1], f32)
                nc.vector.reduce_sum(ss[:], sq[:], axis=X)
                # xinvS = S / ||x||  (sqrt(ss/S^2) then reciprocal)
                t1 = tiny.tile([128, 1], f32)
                nc.scalar.activation(t1[:], ss[:], Act.Sqrt, 0.0, 1.0 / (S * S))
                xinvS = tiny.tile([128, 1], f32)
                nc.vector.reciprocal(xinvS[:], t1[:])
                # xinv = 1 / ||x||
                t2 = tiny.tile([128, 1], f32)
                nc.scalar.activation(t2[:], ss[:], Act.Sqrt)
                nc.vector.reciprocal(xinv8[:, bi : bi + 1], t2[:])
                # xhatS (bf16) and its transpose into xT
                xhS = xsc.tile([128, d], bf16)
                nc.scalar.mul(xhS[:], x_t[:], xinvS[:, :1])
                for k in range(nkt):
                    ptile = pt.tile([128, 128], bf16)
                    nc.tensor.transpose(
                        ptile[:], xhS[:, k * 128 : (k + 1) * 128], ident_bf[:]
                    )
                    col = k * b + bi * 128
                    nc.vector.tensor_copy(xT[:, col : col + 128], ptile[:])
                # wsel row norms + raw dot
                ws_t = xin.tile([128, d], f32)
                nc.sync.dma_start(ws_t[:], wsel_ext[rsl, :])
                sq2 = xsc.tile([128, d], f32)
                nc.vector.tensor_mul(sq2[:], ws_t[:], ws_t[:])
                wss = tiny.tile([128, 1], f32)
                nc.vector.reduce_sum(wss[:], sq2[:], axis=X)
                t3 = tiny.tile([128, 1], f32)
                nc.scalar.activation(t3[:], wss[:], Act.Sqrt)
                nc.vector.reciprocal(wsinv8[:, bi : bi + 1], t3[:])
                pr = xsc.tile([128, d], f32)
                nc.vector.tensor_mul(pr[:], x_t[:], ws_t[:])
                nc.vector.reduce_sum(rawdot8[:, bi : bi + 1], pr[:], axis=X)

            # ---- margin math on [128, nb] ----
            tgt8 = persist.tile([128, nb], f32)
            nc.vector.tensor_mul(tgt8[:], rawdot8[:], xinv8[:])
            nc.vector.tensor_mul(tgt8[:], tgt8[:], wsinv8[:])
            tsq = persist.tile([128, nb], f32)
            nc.vector.tensor_mul(tsq[:], tgt8[:], tgt8[:])
            om = persist.tile([128, nb], f32)
            nc.vector.tensor_scalar(om[:], tsq[:], -1.0, 1.0, Op.mult, Op.add)
            nc.vector.tensor_scalar_max(om[:], om[:], 0.0)
            sine8 = persist.tile([128, nb], f32)
            nc.scalar.activation(sine8[:], om[:], Act.Sqrt)
            phi8 = persist.tile([128, nb], f32)
            nc.vector.tensor_scalar_mul(phi8[:], tgt8[:], COS_M)
            ssin8 = persist.tile([128, nb], f32)
            nc.vector.tensor_scalar_mul(ssin8[:], sine8[:], SIN_M)
            nc.vector.tensor_sub(phi8[:], phi8[:], ssin8[:])
            mask8 = persist.tile([128, nb], mybir.dt.uint8)
            nc.vector.tensor_scalar(mask8[:], tgt8[:], 0.0, None, Op.is_gt)
            selv8 = persist.tile([128, nb], f32)
            nc.vector.select(selv8[:], mask8[:], phi8[:], tgt8[:])
            nc.vector.tensor_scalar_mul(newv8[:], selv8[:], S)
            # flat offsets: i*cs + rel, OOB-marked when rel outside [0, cs)
            o1 = persist.tile([128, nb], f32)
            nc.vector.tensor_add(o1[:], iota_f[:], rel_f[:])
            bad1 = persist.tile([128, nb], f32)
            nc.vector.tensor_scalar(bad1[:], rel_f[:], 0.0, None, Op.is_lt)
            bad2 = persist.tile([128, nb], f32)
            nc.vector.tensor_scalar(bad2[:], rel_f[:], float(cs), None, Op.is_ge)
            nc.vector.tensor_add(bad1[:], bad1[:], bad2[:])
            nc.vector.tensor_scalar_mul(bad1[:], bad1[:], OOB)
            nc.vector.tensor_add(o1[:], o1[:], bad1[:])
            nc.vector.tensor_copy(offs_i[:], o1[:])

            # ---- Phase B: main c-loop ----
            for ci in range(nct):
                csl = slice(ci * tc, (ci + 1) * tc)
                wt_bf_k = []
                wt2_k = []
                for k in range(nkt):
                    wt_f = wstream.tile([128, tc], f32, tag="wt_f")
                    nc.sync.dma_start(wt_f[:], wt_ext[k, :, csl])
                    wt_bf = wbf.tile([128, tc], bf16, tag="wt_bf")
                    nc.gpsimd.tensor_copy(wt_bf[:], wt_f[:])
                    wt2 = wbf.tile([128, tc], bf16, tag="wt2")
                    nc.scalar.square(wt2[:], wt_f[:])
                    wt_bf_k.append(wt_bf)
                    wt2_k.append(wt2)
                nps = pn.tile([128, tc], f32)
                for k in range(nkt):
                    nc.tensor.matmul(
                        nps[:],
                        lhsT=ones_bf[:],
                        rhs=wt2_k[k][:],
                        start=(k == 0),
                        stop=(k == nkt - 1),
                    )
                winv = winvp.tile([128, tc], f32)
                nc.scalar.activation(winv[:], nps[:], Act.Sqrt)
                nc.vector.reciprocal(winv[:], winv[:])
                for bi in range(nb):
                    ops = po.tile([128, tc], f32)
                    for k in range(nkt):
                        col = k * b + bi * 128
                        nc.tensor.matmul(
                            ops[:],
                            lhsT=xT[:, col : col + 128],
                            rhs=wt_bf_k[k][:],
                            start=(k == 0),
                            stop=(k == nkt - 1),
                        )
                    st = stage.tile([128, tc], f32)
                    nc.vector.tensor_tensor(st[:], ops[:], winv[:], Op.mult)
                    nc.sync.dma_start(
                        out_ext[bi * 128 : (bi + 1) * 128, csl], st[:]
                    )

            # ---- Phase C: scatter the margin values ----
            tc_.strict_bb_all_engine_barrier()
            out_flat = out_ext[:].rearrange("r (c one) -> (r c) one", one=1)
            for bi in range(nb):
                nc.gpsimd.indirect_dma_start(
                    out=out_flat,
                    out_offset=bass.IndirectOffsetOnAxis(
                        ap=offs_i[:, bi : bi + 1], axis=0
                    ),
                    in_=newv8[:, bi : bi + 1],
                    in_offset=None,
                    bounds_check=b * cs - 1,
                    oob_is_err=False,
                )

    nc.compile()
    return nc


def host_prep(cfg: Cfg, input, label, weight):
    x = np.ascontiguousarray(np.asarray(input, dtype=np.float32))
    w = np.asarray(weight, dtype=np.float32)
    lab = np.asarray(label).astype(np.int64)
    wsel = np.ascontiguousarray(w[lab])
    wt_all = np.ascontiguousarray(w.T)  # [D, C], relayout only
    in_maps = []
    for core in range(cfg.ncores):
        sl = slice(core * cfg.cs, (core + 1) * cfg.cs)
        wt = np.ascontiguousarray(wt_all[:, sl]).reshape(cfg.nkt, 128, cfg.cs)
        rel = (lab - core * cfg.cs).astype(np.int32)
        labrel = np.ascontiguousarray(rel.reshape(cfg.nb, 128).T)
        in_maps.append({"x": x, "wt": wt, "wsel": wsel, "labrel": labrel})
    return in_maps


def run(cfg: Cfg, nc, in_maps, **kw):
    from concourse.bass_utils import run_bass_kernel_spmd

    res = run_bass_kernel_spmd(nc, in_maps, core_ids=list(range(cfg.ncores)), **kw)
    out = np.concatenate(
        [res.results[c]["out"] for c in range(cfg.ncores)], axis=1
    )
    return out, res


_cache = {}


def kernel(input, label, weight):
    cfg = Cfg()
    if cfg not in _cache:
        _cache[cfg] = build(cfg)
    in_maps = host_prep(cfg, input, label, weight)
    out, _ = run(cfg, _cache[cfg], in_maps)
    return out


# revision 15
# speedup vs baseline: 1.1503x; 1.1503x over previous
"""AAM-Softmax (ArcFace) logits kernel for Trainium2, 8 NeuronCores.

Math (per reference):
    cosine = l2norm(input) @ l2norm(weight).T            # [B, C]
    tgt    = cosine[i, label[i]]
    phi    = tgt*cos(m) - sqrt(1-tgt^2)*sin(m)
    out    = S * cosine, except out[i, label[i]] = S * where(tgt>0, phi, tgt)

Sharding: weight/cosine column-sharded over 8 cores (vocab parallel);
input + labels replicated.  Core k owns classes [k*CS, (k+1)*CS).

Per-core device pipeline:
  - x [B, D] f32 -> row sumsq -> xinvS = S/||x|| (and xinv = 1/||x||)
  - xhatS = x * xinvS (bf16), PE-transposed into xT [D, B] bf16
  - wt input is host-relayouted W.T shard [2, 128, CS] f32 (pure relayout,
    no arithmetic).  Per 500-col tile: cast to bf16; square (bf16) and
    ones-matmul -> column sumsq broadcast over partitions in PSUM;
    sqrt + reciprocal -> winv tile [128, 500].
  - main matmul: out_psum[b-tile] = xT.T @ wt_bf (K=256 over 2 chunks)
  - staging = out_psum * winv  (fuses the weight-norm column scale; x side
    already carries S), DMA to out[b-tile, c-tile].
  - margin: w_sel = weight[label] (host gather, replicated input; all
    arithmetic on device): tgt = (x . wsel) * xinv * wselinv; phi/select
    math on [128, 8]; final values scattered into out[i, label_local[i]]
    via indirect DMA (out-of-shard rows get OOB offsets and are skipped).
"""

import sys

if "/opt/trn_rl_repo" not in sys.path:
    sys.path.insert(0, "/opt/trn_rl_repo")

from dataclasses import dataclass

import numpy as np

S = 50.0
MARGIN = 0.5
COS_M = float(np.cos(MARGIN))
SIN_M = float(np.sin(MARGIN))
OOB = 16000000.0  # exact in f32, > any valid flat offset


@dataclass(frozen=True)
class Cfg:
    b: int = 1024
    d: int = 256
    c: int = 100000
    ncores: int = 8
    tc: int = 500

    @property
    def cs(self):
        return self.c // self.ncores

    @property
    def nb(self):
        return self.b // 128

    @property
    def nkt(self):
        return self.d // 128

    @property
    def nct(self):
        return self.cs // self.tc


def build(cfg: Cfg):
    import concourse.bass as bass
    import concourse.tile as tile
    from concourse import bacc, mybir
    from concourse.masks import make_identity

    f32 = mybir.dt.float32
    bf16 = mybir.dt.bfloat16
    i32 = mybir.dt.int32
    X = mybir.AxisListType.X
    Op = mybir.AluOpType
    Act = mybir.ActivationFunctionType

    b, d, cs, tc = cfg.b, cfg.d, cfg.cs, cfg.tc
    nb, nkt, nct = cfg.nb, cfg.nkt, cfg.nct

    nc = bacc.Bacc(
        "TRN2", target_bir_lowering=False, debug=False, num_devices=cfg.ncores
    )

    x_ext = nc.dram_tensor("x", [b, d], f32, kind="ExternalInput")
    wt_ext = nc.dram_tensor("wt", [nkt, 128, cs], f32, kind="ExternalInput")
    wsel_ext = nc.dram_tensor("wsel", [b, d], f32, kind="ExternalInput")
    labrel_ext = nc.dram_tensor("labrel", [128, nb], i32, kind="ExternalInput")
    out_ext = nc.dram_tensor("out", [b, cs], f32, kind="ExternalOutput")

    # c-tiles are processed in groups; each (b-tile, group) accumulates a
    # wide staging tile so the out DMA moves ncg*tc*4 bytes per partition row
    ncg = min(5, nct)  # c-tiles per group
    assert nct % ncg == 0
    with tile.TileContext(nc) as tc_:
        with (
            tc_.tile_pool(name="const", bufs=1) as constp,
            tc_.tile_pool(name="persist", bufs=1) as persist,
            tc_.tile_pool(name="xin", bufs=2) as xin,
            tc_.tile_pool(name="xsc", bufs=2) as xsc,
            tc_.tile_pool(name="tiny", bufs=2) as tiny,
            tc_.tile_pool(name="wstream", bufs=8) as wstream,
            tc_.tile_pool(name="wbf", bufs=2 * 2 * ncg) as wbf,
            tc_.tile_pool(name="winvp", bufs=3) as winvp,
            tc_.tile_pool(name="stage", bufs=3) as stage,
            tc_.tile_pool(name="pn", bufs=2, space="PSUM") as pn,
            tc_.tile_pool(name="po", bufs=ncg + 1, space="PSUM") as po,
        ):
            ident_bf = constp.tile([128, 128], bf16)
            make_identity(nc, ident_bf[:])
            ones_bf = constp.tile([128, 128], bf16)
            nc.vector.memset(ones_bf[:], 1.0)

            # persistent tensors
            xT = persist.tile([128, nkt * b], bf16)  # [d-half on part][k*b + i]
            labrel_t = persist.tile([128, nb], i32)
            rel_f = persist.tile([128, nb], f32)
            iota_i = persist.tile([128, nb], i32)
            iota_f = persist.tile([128, nb], f32)
            xinv8 = persist.tile([128, nb], f32)
            wsinv8 = persist.tile([128, nb], f32)
            rawdot8 = persist.tile([128, nb], f32)
            newv8 = persist.tile([128, nb], f32)
            offs_i = persist.tile([128, nb], i32)

            nc.sync.dma_start(labrel_t[:], labrel_ext[:])
            # flat row base = p*cs + f*128*cs; iota steps are int16-limited so
            # build p*cs and f separately and combine in f32 (values < 2^24).
            nc.gpsimd.iota(
                iota_i[:], pattern=[[0, nb]], base=0, channel_multiplier=cs
            )
            iota_j = persist.tile([128, nb], i32)
            nc.gpsimd.iota(
                iota_j[:], pattern=[[1, nb]], base=0, channel_multiplier=0
            )
            iotaj_f = persist.tile([128, nb], f32)
            nc.vector.tensor_copy(iota_f[:], iota_i[:])
            nc.vector.tensor_copy(iotaj_f[:], iota_j[:])
            nc.vector.tensor_scalar_mul(iotaj_f[:], iotaj_f[:], float(128 * cs))
            nc.vector.tensor_add(iota_f[:], iota_f[:], iotaj_f[:])
            nc.vector.tensor_copy(rel_f[:], labrel_t[:])

            # ---- Phase A: x prep (+ wsel/tgt path) ----
            for bi in range(nb):
                rsl = slice(bi * 128, (bi + 1) * 128)
                x_t = xin.tile([128, d], f32)
                nc.sync.dma_start(x_t[:], x_ext[rsl, :])
                sq = xsc.tile([128, d], f32)
                nc.vector.tensor_mul(sq[:], x_t[:], x_t[:])
                ss = tiny.tile([128, 1], f32)
                nc.vector.reduce_sum(ss[:], sq[:], axis=X)
                # xinvS = S / ||x||  (sqrt(ss/S^2) then reciprocal)
                t1 = tiny.tile([128, 1], f32)
                nc.scalar.activation(t1[:], ss[:], Act.Sqrt, 0.0, 1.0 / (S * S))
                xinvS = tiny.tile([128, 1], f32)
                nc.vector.reciprocal(xinvS[:], t1[:])
                # xinv = 1 / ||x||
                t2 = tiny.tile([128, 1], f32)
                nc.scalar.activation(t2[:], ss[:], Act.Sqrt)
                nc.vector.reciprocal(xinv8[:, bi : bi + 1], t2[:])
                # xhatS (bf16) and its transpose into xT
                xhS = xsc.tile([128, d], bf16)
                nc.scalar.mul(xhS[:], x_t[:], xinvS[:, :1])
                for k in range(nkt):
                    ptile = po.tile([128, 128], bf16, tag="ops", name="ptile")
                    nc.tensor.transpose(
                        ptile[:], xhS[:, k * 128 : (k + 1) * 128], ident_bf[:]
                    )
                    col = k * b + bi * 128
                    nc.vector.tensor_copy(xT[:, col : col + 128], ptile[:])
                # wsel row norms + raw dot
                ws_t = xin.tile([128, d], f32)
                nc.sync.dma_start(ws_t[:], wsel_ext[rsl, :])
                sq2 = xsc.tile([128, d], f32)
                nc.vector.tensor_mul(sq2[:], ws_t[:], ws_t[:])
                wss = tiny.tile([128, 1], f32)
                nc.vector.reduce_sum(wss[:], sq2[:], axis=X)
                t3 = tiny.tile([128, 1], f32)
                nc.scalar.activation(t3[:], wss[:], Act.Sqrt)
                nc.vector.reciprocal(wsinv8[:, bi : bi + 1], t3[:])
                pr = xsc.tile([128, d], f32)
                nc.vector.tensor_mul(pr[:], x_t[:], ws_t[:])
                nc.vector.reduce_sum(rawdot8[:, bi : bi + 1], pr[:], axis=X)

            # ---- margin math on [128, nb] ----
            tgt8 = persist.tile([128, nb], f32)
            nc.vector.tensor_mul(tgt8[:], rawdot8[:], xinv8[:])
            nc.vector.tensor_mul(tgt8[:], tgt8[:], wsinv8[:])
            tsq = persist.tile([128, nb], f32)
            nc.vector.tensor_mul(tsq[:], tgt8[:], tgt8[:])
            om = persist.tile([128, nb], f32)
            nc.vector.tensor_scalar(om[:], tsq[:], -1.0, 1.0, Op.mult, Op.add)
            nc.vector.tensor_scalar_max(om[:], om[:], 0.0)
            sine8 = persist.tile([128, nb], f32)
            nc.scalar.activation(sine8[:], om[:], Act.Sqrt)
            phi8 = persist.tile([128, nb], f32)
            nc.vector.tensor_scalar_mul(phi8[:], tgt8[:], COS_M)
            ssin8 = persist.tile([128, nb], f32)
            nc.vector.tensor_scalar_mul(ssin8[:], sine8[:], SIN_M)
            nc.vector.tensor_sub(phi8[:], phi8[:], ssin8[:])
            mask8 = persist.tile([128, nb], mybir.dt.uint8)
            nc.vector.tensor_scalar(mask8[:], tgt8[:], 0.0, None, Op.is_gt)
            selv8 = persist.tile([128, nb], f32)
            nc.vector.select(selv8[:], mask8[:], phi8[:], tgt8[:])
            nc.vector.tensor_scalar_mul(newv8[:], selv8[:], S)
            # flat offsets: i*cs + rel, OOB-marked when rel outside [0, cs)
            o1 = persist.tile([128, nb], f32)
            nc.vector.tensor_add(o1[:], iota_f[:], rel_f[:])
            bad1 = persist.tile([128, nb], f32)
            nc.vector.tensor_scalar(bad1[:], rel_f[:], 0.0, None, Op.is_lt)
            bad2 = persist.tile([128, nb], f32)
            nc.vector.tensor_scalar(bad2[:], rel_f[:], float(cs), None, Op.is_ge)
            nc.vector.tensor_add(bad1[:], bad1[:], bad2[:])
            nc.vector.tensor_scalar_mul(bad1[:], bad1[:], OOB)
            nc.vector.tensor_add(o1[:], o1[:], bad1[:])
            nc.vector.tensor_copy(offs_i[:], o1[:])

            # ---- Phase B: main loop over c-groups ----
            for cg in range(nct // ncg):
                # per-group: normalize-scaled bf16 weight tiles
                wt_bf_g = []  # [ci5][k]
                for ci5 in range(ncg):
                    ci = cg * ncg + ci5
                    csl = slice(ci * tc, (ci + 1) * tc)
                    wt_f_k = []
                    wt2_k = []
                    for k in range(nkt):
                        wt_f = wstream.tile([128, tc], f32, tag="wt_f")
                        nc.sync.dma_start(wt_f[:], wt_ext[k, :, csl])
                        wt2 = wstream.tile([128, tc], bf16, tag="wt2")
                        nc.scalar.square(wt2[:], wt_f[:])
                        wt_f_k.append(wt_f)
                        wt2_k.append(wt2)
                    nps = pn.tile([128, tc], f32)
                    for k in range(nkt):
                        nc.tensor.matmul(
                            nps[:],
                            lhsT=ones_bf[:],
                            rhs=wt2_k[k][:],
                            start=(k == 0),
                            stop=(k == nkt - 1),
                        )
                    winv = winvp.tile([128, tc], f32)
                    nc.scalar.activation(winv[:], nps[:], Act.Sqrt)
                    nc.vector.reciprocal(winv[:], winv[:])
                    # fold the column norm into the bf16 weights
                    wt_bf_k = []
                    for k in range(nkt):
                        wt_bf = wbf.tile([128, tc], bf16, tag="wt_bf")
                        nc.vector.tensor_tensor(
                            wt_bf[:], wt_f_k[k][:], winv[:], Op.mult
                        )
                        wt_bf_k.append(wt_bf)
                    wt_bf_g.append(wt_bf_k)
                # matmuls: k-outer keeps the stationary operand loaded
                for bi in range(nb):
                    ops_g = [
                        po.tile([128, tc], f32, tag="ops", name="ops")
                        for _ in range(ncg)
                    ]
                    for k in range(nkt):
                        col = k * b + bi * 128
                        for ci5 in range(ncg):
                            nc.tensor.matmul(
                                ops_g[ci5][:],
                                lhsT=xT[:, col : col + 128],
                                rhs=wt_bf_g[ci5][k][:],
                                start=(k == 0),
                                stop=(k == nkt - 1),
                            )
                    stw = stage.tile([128, ncg * tc], f32)
                    for ci5 in range(ncg):
                        dst = stw[:, ci5 * tc : (ci5 + 1) * tc]
                        if ci5 % 5 < 3:
                            nc.vector.tensor_copy(dst, ops_g[ci5][:])
                        else:
                            nc.scalar.copy(dst, ops_g[ci5][:])
                    nc.sync.dma_start(
                        out_ext[
                            bi * 128 : (bi + 1) * 128,
                            cg * ncg * tc : (cg + 1) * ncg * tc,
                        ],
                        stw[:],
                    )

            # ---- Phase C: scatter the margin values ----
            tc_.strict_bb_all_engine_barrier()
            out_flat = out_ext[:].rearrange("r (c one) -> (r c) one", one=1)
            for bi in range(nb):
                nc.gpsimd.indirect_dma_start(
                    out=out_flat,
                    out_offset=bass.IndirectOffsetOnAxis(
                        ap=offs_i[:, bi : bi + 1], axis=0
                    ),
                    in_=newv8[:, bi : bi + 1],
                    in_offset=None,
                    bounds_check=b * cs - 1,
                    oob_is_err=False,
                )

    nc.compile()
    return nc


def host_prep(cfg: Cfg, input, label, weight):
    x = np.ascontiguousarray(np.asarray(input, dtype=np.float32))
    w = np.asarray(weight, dtype=np.float32)
    lab = np.asarray(label).astype(np.int64)
    wsel = np.ascontiguousarray(w[lab])
    wt_all = np.ascontiguousarray(w.T)  # [D, C], relayout only
    in_maps = []
    for core in range(cfg.ncores):
        sl = slice(core * cfg.cs, (core + 1) * cfg.cs)
        wt = np.ascontiguousarray(wt_all[:, sl]).reshape(cfg.nkt, 128, cfg.cs)
        rel = (lab - core * cfg.cs).astype(np.int32)
        labrel = np.ascontiguousarray(rel.reshape(cfg.nb, 128).T)
        in_maps.append({"x": x, "wt": wt, "wsel": wsel, "labrel": labrel})
    return in_maps


def run(cfg: Cfg, nc, in_maps, **kw):
    from concourse.bass_utils import run_bass_kernel_spmd

    res = run_bass_kernel_spmd(nc, in_maps, core_ids=list(range(cfg.ncores)), **kw)
    out = np.concatenate(
        [res.results[c]["out"] for c in range(cfg.ncores)], axis=1
    )
    return out, res


_cache = {}


def kernel(input, label, weight):
    cfg = Cfg()
    if cfg not in _cache:
        _cache[cfg] = build(cfg)
    in_maps = host_prep(cfg, input, label, weight)
    out, _ = run(cfg, _cache[cfg], in_maps)
    return out


# revision 17
# speedup vs baseline: 1.2448x; 1.0821x over previous
"""AAM-Softmax (ArcFace) logits kernel for Trainium2, 8 NeuronCores.

Math (per reference):
    cosine = l2norm(input) @ l2norm(weight).T            # [B, C]
    tgt    = cosine[i, label[i]]
    phi    = tgt*cos(m) - sqrt(1-tgt^2)*sin(m)
    out    = S * cosine, except out[i, label[i]] = S * where(tgt>0, phi, tgt)

Sharding: weight/cosine column-sharded over 8 cores (vocab parallel);
input + labels replicated.  Core k owns classes [k*CS, (k+1)*CS).

Per-core device pipeline:
  - x [B, D] f32 -> row sumsq -> xinvS = S/||x|| (and xinv = 1/||x||)
  - xhatS = x * xinvS (bf16), PE-transposed into xT [D, B] bf16
  - wt input is host-relayouted W.T shard [2, 128, CS] f32 (pure relayout,
    no arithmetic).  Per 500-col tile: cast to bf16; square (bf16) and
    ones-matmul -> column sumsq broadcast over partitions in PSUM;
    sqrt + reciprocal -> winv tile [128, 500].
  - main matmul: out_psum[b-tile] = xT.T @ wt_bf (K=256 over 2 chunks)
  - staging = out_psum * winv  (fuses the weight-norm column scale; x side
    already carries S), DMA to out[b-tile, c-tile].
  - margin: w_sel = weight[label] (host gather, replicated input; all
    arithmetic on device): tgt = (x . wsel) * xinv * wselinv; phi/select
    math on [128, 8]; final values scattered into out[i, label_local[i]]
    via indirect DMA (out-of-shard rows get OOB offsets and are skipped).
"""

import sys

if "/opt/trn_rl_repo" not in sys.path:
    sys.path.insert(0, "/opt/trn_rl_repo")

from dataclasses import dataclass

import numpy as np

S = 50.0
MARGIN = 0.5
COS_M = float(np.cos(MARGIN))
SIN_M = float(np.sin(MARGIN))
OOB = 16000000.0  # exact in f32, > any valid flat offset


@dataclass(frozen=True)
class Cfg:
    b: int = 1024
    d: int = 256
    c: int = 100000
    ncores: int = 8
    tc: int = 500

    @property
    def cs(self):
        return self.c // self.ncores

    @property
    def nb(self):
        return self.b // 128

    @property
    def nkt(self):
        return self.d // 128

    @property
    def nct(self):
        return self.cs // self.tc


def build(cfg: Cfg):
    import concourse.bass as bass
    import concourse.tile as tile
    from concourse import bacc, mybir
    from concourse.masks import make_identity

    f32 = mybir.dt.float32
    bf16 = mybir.dt.bfloat16
    i32 = mybir.dt.int32
    X = mybir.AxisListType.X
    Op = mybir.AluOpType
    Act = mybir.ActivationFunctionType

    b, d, cs, tc = cfg.b, cfg.d, cfg.cs, cfg.tc
    nb, nkt, nct = cfg.nb, cfg.nkt, cfg.nct

    nc = bacc.Bacc(
        "TRN2", target_bir_lowering=False, debug=False, num_devices=cfg.ncores
    )

    x_ext = nc.dram_tensor("x", [b, d], f32, kind="ExternalInput")
    wt_ext = nc.dram_tensor("wt", [nkt, 128, cs], f32, kind="ExternalInput")
    wsel_ext = nc.dram_tensor("wsel", [b, d], f32, kind="ExternalInput")
    labrel_ext = nc.dram_tensor("labrel", [128, nb], i32, kind="ExternalInput")
    out_ext = nc.dram_tensor("out", [b, cs], f32, kind="ExternalOutput")

    # c-tiles are processed in groups; each (b-tile, group) accumulates a
    # wide staging tile so the out DMA moves ncg*tc*4 bytes per partition row
    ncg = min(5, nct)  # c-tiles per group
    assert nct % ncg == 0
    with tile.TileContext(nc) as tc_:
        with (
            tc_.tile_pool(name="const", bufs=1) as constp,
            tc_.tile_pool(name="persist", bufs=1) as persist,
            tc_.tile_pool(name="xin", bufs=2) as xin,
            tc_.tile_pool(name="xsc", bufs=2) as xsc,
            tc_.tile_pool(name="tiny", bufs=2) as tiny,
            tc_.tile_pool(name="wstream", bufs=8) as wstream,
            tc_.tile_pool(name="wbf", bufs=2 * 2 * ncg) as wbf,
            tc_.tile_pool(name="winvp", bufs=3) as winvp,
            tc_.tile_pool(name="stage", bufs=4) as stage,
            tc_.tile_pool(name="pn", bufs=2, space="PSUM") as pn,
            tc_.tile_pool(name="po", bufs=ncg + 1, space="PSUM") as po,
        ):
            ident_bf = constp.tile([128, 128], bf16)
            make_identity(nc, ident_bf[:])
            ones_bf = constp.tile([128, 128], bf16)
            nc.vector.memset(ones_bf[:], 1.0)

            # persistent tensors
            xT = persist.tile([128, nkt * b], bf16)  # [d-half on part][k*b + i]
            labrel_t = persist.tile([128, nb], i32)
            rel_f = persist.tile([128, nb], f32)
            iota_i = persist.tile([128, nb], i32)
            iota_f = persist.tile([128, nb], f32)
            xinv8 = persist.tile([128, nb], f32)
            wsinv8 = persist.tile([128, nb], f32)
            rawdot8 = persist.tile([128, nb], f32)
            newv8 = persist.tile([128, nb], f32)
            offs_i = persist.tile([128, nb], i32)

            nc.sync.dma_start(labrel_t[:], labrel_ext[:])
            # flat row base = p*cs + f*128*cs; iota steps are int16-limited so
            # build p*cs and f separately and combine in f32 (values < 2^24).
            nc.gpsimd.iota(
                iota_i[:], pattern=[[0, nb]], base=0, channel_multiplier=cs
            )
            iota_j = persist.tile([128, nb], i32)
            nc.gpsimd.iota(
                iota_j[:], pattern=[[1, nb]], base=0, channel_multiplier=0
            )
            iotaj_f = persist.tile([128, nb], f32)
            nc.vector.tensor_copy(iota_f[:], iota_i[:])
            nc.vector.tensor_copy(iotaj_f[:], iota_j[:])
            nc.vector.tensor_scalar_mul(iotaj_f[:], iotaj_f[:], float(128 * cs))
            nc.vector.tensor_add(iota_f[:], iota_f[:], iotaj_f[:])
            nc.vector.tensor_copy(rel_f[:], labrel_t[:])

            # ---- Phase A: x prep (+ wsel/tgt path) ----
            for bi in range(nb):
                rsl = slice(bi * 128, (bi + 1) * 128)
                x_t = xin.tile([128, d], f32)
                nc.sync.dma_start(x_t[:], x_ext[rsl, :])
                sq = xsc.tile([128, d], f32)
                nc.vector.tensor_mul(sq[:], x_t[:], x_t[:])
                ss = tiny.tile([128, 1], f32)
                nc.vector.reduce_sum(ss[:], sq[:], axis=X)
                # xinvS = S / ||x||  (sqrt(ss/S^2) then reciprocal)
                t1 = tiny.tile([128, 1], f32)
                nc.scalar.activation(t1[:], ss[:], Act.Sqrt, 0.0, 1.0 / (S * S))
                xinvS = tiny.tile([128, 1], f32)
                nc.vector.reciprocal(xinvS[:], t1[:])
                # xinv = 1 / ||x||
                t2 = tiny.tile([128, 1], f32)
                nc.scalar.activation(t2[:], ss[:], Act.Sqrt)
                nc.vector.reciprocal(xinv8[:, bi : bi + 1], t2[:])
                # xhatS (bf16) and its transpose into xT
                xhS = xsc.tile([128, d], bf16)
                nc.scalar.mul(xhS[:], x_t[:], xinvS[:, :1])
                for k in range(nkt):
                    ptile = po.tile([128, 128], bf16, tag="ops", name="ptile")
                    nc.tensor.transpose(
                        ptile[:], xhS[:, k * 128 : (k + 1) * 128], ident_bf[:]
                    )
                    col = k * b + bi * 128
                    nc.vector.tensor_copy(xT[:, col : col + 128], ptile[:])
                # wsel row norms + raw dot
                ws_t = xin.tile([128, d], f32)
                nc.sync.dma_start(ws_t[:], wsel_ext[rsl, :])
                sq2 = xsc.tile([128, d], f32)
                nc.vector.tensor_mul(sq2[:], ws_t[:], ws_t[:])
                wss = tiny.tile([128, 1], f32)
                nc.vector.reduce_sum(wss[:], sq2[:], axis=X)
                t3 = tiny.tile([128, 1], f32)
                nc.scalar.activation(t3[:], wss[:], Act.Sqrt)
                nc.vector.reciprocal(wsinv8[:, bi : bi + 1], t3[:])
                pr = xsc.tile([128, d], f32)
                nc.vector.tensor_mul(pr[:], x_t[:], ws_t[:])
                nc.vector.reduce_sum(rawdot8[:, bi : bi + 1], pr[:], axis=X)

            # ---- margin math on [128, nb] ----
            tgt8 = persist.tile([128, nb], f32)
            nc.vector.tensor_mul(tgt8[:], rawdot8[:], xinv8[:])
            nc.vector.tensor_mul(tgt8[:], tgt8[:], wsinv8[:])
            tsq = persist.tile([128, nb], f32)
            nc.vector.tensor_mul(tsq[:], tgt8[:], tgt8[:])
            om = persist.tile([128, nb], f32)
            nc.vector.tensor_scalar(om[:], tsq[:], -1.0, 1.0, Op.mult, Op.add)
            nc.vector.tensor_scalar_max(om[:], om[:], 0.0)
            sine8 = persist.tile([128, nb], f32)
            nc.scalar.activation(sine8[:], om[:], Act.Sqrt)
            phi8 = persist.tile([128, nb], f32)
            nc.vector.tensor_scalar_mul(phi8[:], tgt8[:], COS_M)
            ssin8 = persist.tile([128, nb], f32)
            nc.vector.tensor_scalar_mul(ssin8[:], sine8[:], SIN_M)
            nc.vector.tensor_sub(phi8[:], phi8[:], ssin8[:])
            mask8 = persist.tile([128, nb], mybir.dt.uint8)
            nc.vector.tensor_scalar(mask8[:], tgt8[:], 0.0, None, Op.is_gt)
            selv8 = persist.tile([128, nb], f32)
            nc.vector.select(selv8[:], mask8[:], phi8[:], tgt8[:])
            nc.vector.tensor_scalar_mul(newv8[:], selv8[:], S)
            # flat offsets: i*cs + rel, OOB-marked when rel outside [0, cs)
            o1 = persist.tile([128, nb], f32)
            nc.vector.tensor_add(o1[:], iota_f[:], rel_f[:])
            bad1 = persist.tile([128, nb], f32)
            nc.vector.tensor_scalar(bad1[:], rel_f[:], 0.0, None, Op.is_lt)
            bad2 = persist.tile([128, nb], f32)
            nc.vector.tensor_scalar(bad2[:], rel_f[:], float(cs), None, Op.is_ge)
            nc.vector.tensor_add(bad1[:], bad1[:], bad2[:])
            nc.vector.tensor_scalar_mul(bad1[:], bad1[:], OOB)
            nc.vector.tensor_add(o1[:], o1[:], bad1[:])
            nc.vector.tensor_copy(offs_i[:], o1[:])

            # ---- Phase B: main loop over c-groups ----
            for cg in range(nct // ncg):
                # per-group: normalize-scaled bf16 weight tiles
                wt_bf_g = []  # [ci5][k]
                for ci5 in range(ncg):
                    ci = cg * ncg + ci5
                    csl = slice(ci * tc, (ci + 1) * tc)
                    wt_f_k = []
                    wt2_k = []
                    for k in range(nkt):
                        wt_f = wstream.tile([128, tc], f32, tag="wt_f")
                        nc.sync.dma_start(wt_f[:], wt_ext[k, :, csl])
                        wt2 = wstream.tile([128, tc], bf16, tag="wt2")
                        nc.scalar.square(wt2[:], wt_f[:])
                        wt_f_k.append(wt_f)
                        wt2_k.append(wt2)
                    nps = pn.tile([128, tc], f32)
                    for k in range(nkt):
                        nc.tensor.matmul(
                            nps[:],
                            lhsT=ones_bf[:],
                            rhs=wt2_k[k][:],
                            start=(k == 0),
                            stop=(k == nkt - 1),
                        )
                    # winv = n2^(-1/2) = exp(-0.5*ln(n2)) — ACT-only, keeps
                    # the expensive DVE reciprocal off the critical engine
                    wlog = winvp.tile([128, tc], f32, tag="wlog")
                    nc.scalar.activation(wlog[:], nps[:], Act.Ln)
                    winv = winvp.tile([128, tc], f32)
                    nc.scalar.activation(winv[:], wlog[:], Act.Exp, 0.0, -0.5)
                    # fold the column norm into the bf16 weights
                    wt_bf_k = []
                    for k in range(nkt):
                        wt_bf = wbf.tile([128, tc], bf16, tag="wt_bf")
                        nc.vector.tensor_tensor(
                            wt_bf[:], wt_f_k[k][:], winv[:], Op.mult
                        )
                        wt_bf_k.append(wt_bf)
                    wt_bf_g.append(wt_bf_k)
                # matmuls: k-outer keeps the stationary operand loaded
                for bi in range(nb):
                    ops_g = [
                        po.tile([128, tc], f32, tag="ops", name="ops")
                        for _ in range(ncg)
                    ]
                    for k in range(nkt):
                        col = k * b + bi * 128
                        for ci5 in range(ncg):
                            nc.tensor.matmul(
                                ops_g[ci5][:],
                                lhsT=xT[:, col : col + 128],
                                rhs=wt_bf_g[ci5][k][:],
                                start=(k == 0),
                                stop=(k == nkt - 1),
                            )
                    stw = stage.tile([128, ncg * tc], f32)
                    for ci5 in range(ncg):
                        dst = stw[:, ci5 * tc : (ci5 + 1) * tc]
                        if ci5 % 5 < 3:
                            nc.vector.tensor_copy(dst, ops_g[ci5][:])
                        else:
                            nc.scalar.copy(dst, ops_g[ci5][:])
                    nc.sync.dma_start(
                        out_ext[
                            bi * 128 : (bi + 1) * 128,
                            cg * ncg * tc : (cg + 1) * ncg * tc,
                        ],
                        stw[:],
                    )

            # ---- Phase C: scatter the margin values ----
            tc_.strict_bb_all_engine_barrier()
            out_flat = out_ext[:].rearrange("r (c one) -> (r c) one", one=1)
            for bi in range(nb):
                nc.gpsimd.indirect_dma_start(
                    out=out_flat,
                    out_offset=bass.IndirectOffsetOnAxis(
                        ap=offs_i[:, bi : bi + 1], axis=0
                    ),
                    in_=newv8[:, bi : bi + 1],
                    in_offset=None,
                    bounds_check=b * cs - 1,
                    oob_is_err=False,
                )

    nc.compile()
    return nc


def host_prep(cfg: Cfg, input, label, weight):
    x = np.ascontiguousarray(np.asarray(input, dtype=np.float32))
    w = np.asarray(weight, dtype=np.float32)
    lab = np.asarray(label).astype(np.int64)
    wsel = np.ascontiguousarray(w[lab])
    wt_all = np.ascontiguousarray(w.T)  # [D, C], relayout only
    in_maps = []
    for core in range(cfg.ncores):
        sl = slice(core * cfg.cs, (core + 1) * cfg.cs)
        wt = np.ascontiguousarray(wt_all[:, sl]).reshape(cfg.nkt, 128, cfg.cs)
        rel = (lab - core * cfg.cs).astype(np.int32)
        labrel = np.ascontiguousarray(rel.reshape(cfg.nb, 128).T)
        in_maps.append({"x": x, "wt": wt, "wsel": wsel, "labrel": labrel})
    return in_maps


def run(cfg: Cfg, nc, in_maps, **kw):
    from concourse.bass_utils import run_bass_kernel_spmd

    res = run_bass_kernel_spmd(nc, in_maps, core_ids=list(range(cfg.ncores)), **kw)
    out = np.concatenate(
        [res.results[c]["out"] for c in range(cfg.ncores)], axis=1
    )
    return out, res


_cache = {}


def kernel(input, label, weight):
    cfg = Cfg()
    if cfg not in _cache:
        _cache[cfg] = build(cfg)
    in_maps = host_prep(cfg, input, label, weight)
    out, _ = run(cfg, _cache[cfg], in_maps)
    return out
